# revision 45
# baseline (speedup 1.0000x reference)
"""Trainium2 Bass kernel for ContextAwareRegionalAttentionNetwork.

Computes, for B=4 images of [C=2048, 80, 80] features and R=2000 ROIs:
  roi_mean[r, c]  = mean of features[b_r, c] over the ROI window
  pooled[r]       = concat(roi_mean[r], gmean[b_r])            # [2C]
  out[0, r]       = softplus(W2 @ tanh(W1 @ pooled[r] + b1) + b2)

Strategy (8 NeuronCores, image x y-half sharded, projection-first):
  - Everything before tanh is linear in the features, so the memory-bound
    bulk of the work is a projection of the 2048 channels down to the 64
    MLP hidden channels on the TensorEngine:
    P[o, y, x] = sum_c W1a[o, c] * feat[c, y, x]   (210 MB -> 3.3 MB).
  - core k owns image k//2, y-half k%2 (40 rows) as fp8-e4m3 (6.55 MB —
    host converts; end-to-end output error ~5e-3 vs the 2e-2 gate).  The
    projection runs in fp8 DoubleRow mode (two 128-channel k-tiles per
    pass), which fills the 128-wide PE array despite only 64 outputs.
    No inter-core collectives.
  - The half streams in as 8 full-width superblock DMAs (contiguous 6400B
    lines; superblock 0 split so the PE starts early); 7 x 512-column PSUM
    chunks accumulate P, vector/scalar copies drain them to SBUF, and the
    sync queue (idle after the feature stream) DMAs P out.  Device tail
    after the last matmul is ~2 us.
  - The host builds the per-region summed-area tables from P (np.cumsum on
    3.3 MB, a fraction of a ms), gathers the 4 SAT corners per (ROI,
    region) with fancy indexing, and finishes with recip/area scaling, the
    global-context term, b1, and the tiny tanh/W2/softplus MLP.  Device-
    side gathers are impractical: gpsimd ap_gather costs ~27 ns of hidden
    Q7 time per index (~56 us per 2080-index gather, measured); and the
    device-side SAT scans (DVE tensor_tensor_scan + transpose) put ~6 us
    of serial chain on the kernel tail for work that is 0.2% of the FLOPs.
"""

import numpy as np
import ml_dtypes
from contextlib import ExitStack

import concourse.bass as bass
import concourse.tile as tile
from concourse import bacc, mybir
from concourse.bass_utils import run_bass_kernel_spmd

f32 = mybir.dt.float32

# feature/weight dtype on device: fp8-e4m3 halves HBM traffic vs fp16 and
# keeps the end-to-end output error ~5e-3, well under the 2e-2 gate.
FEAT_DT = mybir.dt.float8e4
FEAT_NP = ml_dtypes.float8_e4m3

B, C, H, W = 4, 2048, 80, 80
R = 2000
SCALE = 0.03125
NCORES = 8
NSB = C // 256             # 8 channel superblocks (2 k-tiles each, DoubleRow)
HALF_ROWS = H // 2         # 40 rows per core

RHS = (10, 10, 10, 10)     # rows per region; pairs: (0,1) and (2,3)
NREG = len(RHS)
RPXS = tuple(rh * W for rh in RHS)            # pixels per region
ROFF = tuple(int(np.cumsum((0,) + RPXS)[q]) for q in range(NREG + 1))
HPX = H * W // 2           # 3200 pixels per half
PAIRS = ((0, 1), (2, 3))


def _host_prep(rois):
    """Decode ROIs exactly like the reference."""
    rois = np.asarray(rois, np.float32)
    b = rois[:, 0].astype(np.int32)
    coords = np.round(rois[:, 1:] * np.float32(SCALE)).astype(np.int32)
    x1, y1, x2, y2 = coords[:, 0], coords[:, 1], coords[:, 2], coords[:, 3]
    rw = np.maximum(x2 - x1 + 1, 1)
    rh = np.maximum(y2 - y1 + 1, 1)
    hs = np.clip(y1, 0, H)
    he = np.clip(y1 + rh, 0, H)
    ws = np.clip(x1, 0, W)
    we = np.clip(x1 + rw, 0, W)
    area = ((he - hs) * (we - ws)).astype(np.float32)
    empty = (he <= hs) | (we <= ws)
    recip = np.where(empty, 0.0, 1.0 / np.maximum(area, 1.0)).astype(np.float32)
    groups = [np.nonzero(b == img)[0] for img in range(B)]
    return groups, hs, he, ws, we, recip, empty


def _build():
    nc = bacc.Bacc("TRN2", target_bir_lowering=False, debug=False,
                   num_devices=1)
    # feat rows: superblock sb holds channels 256*sb..256*sb+255 as
    # [128 partitions, (pair=2, ktile=2, pair-pixels)] so each region-pair
    # DMA line is fully contiguous; wt free layout is (sb, ktile, o).
    feat_d = nc.dram_tensor("feat", [NSB * 128, 2 * HPX], FEAT_DT,
                            kind="ExternalInput").ap()
    wt_d = nc.dram_tensor("wt", [128, NSB * 128], FEAT_DT,
                          kind="ExternalInput").ap()
    p_d = nc.dram_tensor("p", [64, HPX], f32, kind="ExternalOutput").ap()

    dr = mybir.MatmulPerfMode.DoubleRow

    with tile.TileContext(nc) as tc, ExitStack() as ctx:
        const = ctx.enter_context(tc.tile_pool(name="const", bufs=1))
        fpool = ctx.enter_context(tc.tile_pool(name="feat", bufs=7))
        rpool = ctx.enter_context(tc.tile_pool(name="reg", bufs=1))
        ppool = ctx.enter_context(tc.tile_pool(name="ps", bufs=1, space="PSUM"))

        # constants (scalar HWDGE queue, so feature DMAs start immediately);
        # superblock 0's weights load first (tiny) so matmuls start early
        wt0 = const.tile([128, 128], FEAT_DT)
        nc.scalar.dma_start(wt0[:], wt_d[:, 0:128])
        wtr = const.tile([128, (NSB - 1) * 128], FEAT_DT)
        nc.scalar.dma_start(wtr[:], wt_d[:, 128:])

        def wt_view(sb):
            t = wt0[:] if sb == 0 else wtr[:, 128 * (sb - 1):128 * sb]
            return t.rearrange("p (two m) -> p two m", two=2)

        # PSUM chunking is decoupled from the host's row-aligned regions:
        # 7 column chunks (6x512 + 128) minimize matmul count (56 total).
        CH = [(512 * i, min(512 * (i + 1), HPX)) for i in range((HPX + 511) // 512)]

        # feature superblocks stream as pieces sized so data lands in matmul
        # consumption order: the PE never waits a whole 819KB superblock.
        # Piece boundaries (in pixels) align with 512-col PSUM chunks; the
        # tuple maps chunk index -> (piece index, pixel offset of piece).
        fts = []
        f3 = feat_d.rearrange("c (two n) -> c two n", two=2)
        SPLITS = {0: (0, 512, 1024, 1536, 2048, HPX), None: (0, 2048, HPX)}
        CMAP = {}
        for sb in range(NSB):
            bounds = SPLITS[0] if sb == 0 else SPLITS[None]
            pieces = []
            for pi in range(len(bounds) - 1):
                n0, n1 = bounds[pi], bounds[pi + 1]
                t = fpool.tile([128, 2 * (n1 - n0)], FEAT_DT,
                               tag=f"ft{min(sb,1)}_{pi}", name=f"f{sb}_{pi}",
                               bufs=1 if sb == 0 else 7)
                eng = nc.sync if (sb + pi) % 2 == 0 else nc.scalar
                eng.dma_start(
                    t[:].rearrange("p (two n) -> p two n", two=2),
                    f3[128 * sb:128 * (sb + 1), :, n0:n1])
                pieces.append((t, n0))
            fts.append(pieces)
            if sb <= 1:
                CMAP[sb] = [max(pi for pi in range(len(bounds) - 1)
                                if bounds[pi] <= c0) for c0, _ in CH]

        pss = [ppool.tile([64, c1 - c0], f32, tag=f"ps{ci}", name=f"ps{ci}",
                          padded_shape=[64, 512]) for ci, (c0, c1) in enumerate(CH)]
        for sb in range(NSB):
            cmap = CMAP[min(sb, 1)]
            for ci, (c0, c1) in enumerate(CH):
                tl, n0 = fts[sb][cmap[ci]]
                ft3 = tl[:].rearrange("p (two n) -> p two n", two=2)
                nc.tensor.matmul(pss[ci][:, 0:c1 - c0], wt_view(sb),
                                 ft3[:, :, c0 - n0:c1 - n0],
                                 start=(sb == 0), stop=(sb == NSB - 1),
                                 perf_mode=dr)
        # PSUM -> SBUF (alternate vector/scalar; chunk stops stagger across
        # the last superblock's matmuls), then DMA out on the sync queue,
        # which is done streaming features by now
        for ci, (c0, c1) in enumerate(CH):
            pt = rpool.tile([64, c1 - c0], f32, tag=f"pt{ci}", name=f"pt{ci}")
            if ci % 2 == 0:
                nc.vector.tensor_copy(pt[:], pss[ci][:])
            else:
                nc.scalar.copy(pt[:], pss[ci][:])
            nc.sync.dma_start(p_d[:, c0:c1], pt[:])
    nc.compile()
    return nc


_CACHE = {}


def _get_program():
    if "nc" not in _CACHE:
        _CACHE["nc"] = _build()
    return _CACHE["nc"]


def kernel(features, rois, W1, b1, W2, b2, _want_trace=False, _trace_kwargs=None):
    features = np.asarray(features, np.float32)
    W1 = np.asarray(W1, np.float32)
    b1 = np.asarray(b1, np.float32).reshape(64)
    W2 = np.asarray(W2, np.float32).reshape(1, 64)
    b2 = np.asarray(b2, np.float32).reshape(1)

    groups, hs, he, ws, we, recip, empty = _host_prep(rois)
    nc = _get_program()

    feat8 = features.astype(FEAT_NP)
    # wt free layout (sb, ktile, o): wt[p, 128*sb + 64*i + m] = W1[m, 256sb+128i+p]
    wt = np.ascontiguousarray(
        W1[:, :C].T.reshape(NSB, 2, 128, 64).transpose(2, 0, 1, 3)
        .reshape(128, NSB * 128)
    ).astype(FEAT_NP)

    in_maps = []
    for k in range(NCORES):
        img, hlf = k // 2, k % 2
        feat_k = feat8[img, :, hlf * HALF_ROWS:(hlf + 1) * HALF_ROWS, :]
        # [2048, 3200] -> [sb, ktile, 128p, n] -> [sb, p, (ktile, n)] rows:
        # each partition line is one contiguous 6400B DMA segment
        feat_k = (feat_k.reshape(NSB, 2, 128, HPX)
                  .transpose(0, 2, 1, 3).reshape(NSB * 128, 2 * HPX))
        in_maps.append({
            "feat": np.ascontiguousarray(feat_k),
            "wt": wt,
        })
    res = run_bass_kernel_spmd(nc, in_maps, list(range(NCORES)),
                               trace=_want_trace, **(_trace_kwargs or {}))

    # host epilogue: build per-region SATs from projected P, gather the 4
    # corners per (ROI, region), sum regions, scale by 1/area, add the
    # global-context term + b1, then tanh / W2 / softplus.
    gmean = features.mean(axis=(2, 3))          # [B, C]
    gterm = gmean @ W1[:, C:].T                 # [B, 64]
    out = np.zeros((1, R), np.float32)
    for img in range(B):
        g = groups[img]
        n = len(g)
        win = np.zeros((64, n), np.float32)
        for k in (2 * img, 2 * img + 1):
            hlf = k % 2
            pflat = np.asarray(res.results[k]["p"])         # [64, HPX]
            row0 = hlf * HALF_ROWS
            for q in range(NREG):
                rh = RHS[q]
                P = pflat[:, ROFF[q]:ROFF[q + 1]].reshape(64, rh, W)
                S = np.zeros((64, rh + 1, W + 1), np.float32)
                np.cumsum(np.cumsum(P, axis=2), axis=1, out=S[:, 1:, 1:])
                r0 = row0 + sum(RHS[:q])
                ls = np.clip(hs[g] - r0, 0, rh)
                le = np.clip(he[g] - r0, 0, rh)
                win += (S[:, le, we[g]] - S[:, ls, we[g]]
                        - S[:, le, ws[g]] + S[:, ls, ws[g]])
        pre = win * recip[g][None, :] + b1[:, None]
        pre = pre + np.where(empty[g][None, :], 0.0, gterm[img][:, None])
        h = np.tanh(pre)
        kk = W2 @ h + b2[:, None]               # [1, n]
        out[0, g] = np.log1p(np.exp(kk[0]))
    if _want_trace:
        return out, res
    return out


# revision 46
# speedup vs baseline: 1.0109x; 1.0109x over previous
"""Trainium2 Bass kernel for ContextAwareRegionalAttentionNetwork.

Computes, for B=4 images of [C=2048, 80, 80] features and R=2000 ROIs:
  roi_mean[r, c]  = mean of features[b_r, c] over the ROI window
  pooled[r]       = concat(roi_mean[r], gmean[b_r])            # [2C]
  out[0, r]       = softplus(W2 @ tanh(W1 @ pooled[r] + b1) + b2)

Strategy (8 NeuronCores, image x y-half sharded, projection-first):
  - Everything before tanh is linear in the features, so the memory-bound
    bulk of the work is a projection of the 2048 channels down to the 64
    MLP hidden channels on the TensorEngine:
    P[o, y, x] = sum_c W1a[o, c] * feat[c, y, x]   (210 MB -> 3.3 MB).
  - core k owns image k//2, y-half k%2 (40 rows) as fp8-e4m3 (6.55 MB —
    host converts; end-to-end output error ~5e-3 vs the 2e-2 gate).  The
    projection runs in fp8 DoubleRow mode (two 128-channel k-tiles per
    pass), which fills the 128-wide PE array despite only 64 outputs.
    No inter-core collectives.
  - The half streams in as 8 full-width superblock DMAs (contiguous 6400B
    lines; superblock 0 split so the PE starts early); 7 x 512-column PSUM
    chunks accumulate P, vector/scalar copies drain them to SBUF, and the
    sync queue (idle after the feature stream) DMAs P out.  Device tail
    after the last matmul is ~2 us.
  - The host builds the per-region summed-area tables from P (np.cumsum on
    3.3 MB, a fraction of a ms), gathers the 4 SAT corners per (ROI,
    region) with fancy indexing, and finishes with recip/area scaling, the
    global-context term, b1, and the tiny tanh/W2/softplus MLP.  Device-
    side gathers are impractical: gpsimd ap_gather costs ~27 ns of hidden
    Q7 time per index (~56 us per 2080-index gather, measured); and the
    device-side SAT scans (DVE tensor_tensor_scan + transpose) put ~6 us
    of serial chain on the kernel tail for work that is 0.2% of the FLOPs.
"""

import numpy as np
import ml_dtypes
from contextlib import ExitStack

import concourse.bass as bass
import concourse.tile as tile
from concourse import bacc, mybir
from concourse.bass_utils import run_bass_kernel_spmd

f32 = mybir.dt.float32

# feature/weight dtype on device: fp8-e4m3 halves HBM traffic vs fp16 and
# keeps the end-to-end output error ~5e-3, well under the 2e-2 gate.
FEAT_DT = mybir.dt.float8e4
FEAT_NP = ml_dtypes.float8_e4m3

B, C, H, W = 4, 2048, 80, 80
R = 2000
SCALE = 0.03125
NCORES = 8
NSB = C // 256             # 8 channel superblocks (2 k-tiles each, DoubleRow)
HALF_ROWS = H // 2         # 40 rows per core

RHS = (10, 10, 10, 10)     # rows per region; pairs: (0,1) and (2,3)
NREG = len(RHS)
RPXS = tuple(rh * W for rh in RHS)            # pixels per region
ROFF = tuple(int(np.cumsum((0,) + RPXS)[q]) for q in range(NREG + 1))
HPX = H * W // 2           # 3200 pixels per half
PAIRS = ((0, 1), (2, 3))


def _host_prep(rois):
    """Decode ROIs exactly like the reference."""
    rois = np.asarray(rois, np.float32)
    b = rois[:, 0].astype(np.int32)
    coords = np.round(rois[:, 1:] * np.float32(SCALE)).astype(np.int32)
    x1, y1, x2, y2 = coords[:, 0], coords[:, 1], coords[:, 2], coords[:, 3]
    rw = np.maximum(x2 - x1 + 1, 1)
    rh = np.maximum(y2 - y1 + 1, 1)
    hs = np.clip(y1, 0, H)
    he = np.clip(y1 + rh, 0, H)
    ws = np.clip(x1, 0, W)
    we = np.clip(x1 + rw, 0, W)
    area = ((he - hs) * (we - ws)).astype(np.float32)
    empty = (he <= hs) | (we <= ws)
    recip = np.where(empty, 0.0, 1.0 / np.maximum(area, 1.0)).astype(np.float32)
    groups = [np.nonzero(b == img)[0] for img in range(B)]
    return groups, hs, he, ws, we, recip, empty


def _build():
    nc = bacc.Bacc("TRN2", target_bir_lowering=False, debug=False,
                   num_devices=1)
    # feat rows: superblock sb holds channels 256*sb..256*sb+255 as
    # [128 partitions, (pair=2, ktile=2, pair-pixels)] so each region-pair
    # DMA line is fully contiguous; wt free layout is (sb, ktile, o).
    feat_d = nc.dram_tensor("feat", [NSB * 128, 2 * HPX], FEAT_DT,
                            kind="ExternalInput").ap()
    wt_d = nc.dram_tensor("wt", [128, NSB * 128], FEAT_DT,
                          kind="ExternalInput").ap()
    p_d = nc.dram_tensor("p", [64, HPX], f32, kind="ExternalOutput").ap()

    dr = mybir.MatmulPerfMode.DoubleRow

    with tile.TileContext(nc) as tc, ExitStack() as ctx:
        const = ctx.enter_context(tc.tile_pool(name="const", bufs=1))
        fpool = ctx.enter_context(tc.tile_pool(name="feat", bufs=7))
        rpool = ctx.enter_context(tc.tile_pool(name="reg", bufs=1))
        ppool = ctx.enter_context(tc.tile_pool(name="ps", bufs=1, space="PSUM"))

        # constants (scalar HWDGE queue, so feature DMAs start immediately);
        # superblock 0's weights load first (tiny) so matmuls start early
        wt0 = const.tile([128, 128], FEAT_DT)
        nc.scalar.dma_start(wt0[:], wt_d[:, 0:128])
        wtr = const.tile([128, (NSB - 1) * 128], FEAT_DT)
        nc.scalar.dma_start(wtr[:], wt_d[:, 128:])

        def wt_view(sb):
            t = wt0[:] if sb == 0 else wtr[:, 128 * (sb - 1):128 * sb]
            return t.rearrange("p (two m) -> p two m", two=2)

        # PSUM chunking is decoupled from the host's row-aligned regions:
        # 7 column chunks (6x512 + 128) minimize matmul count (56 total).
        CH = [(512 * i, min(512 * (i + 1), HPX)) for i in range((HPX + 511) // 512)]

        # feature superblocks stream as pieces sized so data lands in matmul
        # consumption order: the PE never waits a whole 819KB superblock.
        # Piece boundaries (in pixels) align with 512-col PSUM chunks; the
        # tuple maps chunk index -> (piece index, pixel offset of piece).
        fts = []
        f3 = feat_d.rearrange("c (two n) -> c two n", two=2)
        SPLITS = {0: (0, 512, 1024, 1536, 2048, HPX), None: (0, 2048, HPX)}
        CMAP = {}
        for sb in range(NSB):
            bounds = SPLITS[0] if sb == 0 else SPLITS[None]
            pieces = []
            for pi in range(len(bounds) - 1):
                n0, n1 = bounds[pi], bounds[pi + 1]
                t = fpool.tile([128, 2 * (n1 - n0)], FEAT_DT,
                               tag=f"ft{min(sb,1)}_{pi}", name=f"f{sb}_{pi}",
                               bufs=1 if sb == 0 else 7)
                eng = nc.sync if (sb + pi) % 2 == 0 else nc.scalar
                eng.dma_start(
                    t[:].rearrange("p (two n) -> p two n", two=2),
                    f3[128 * sb:128 * (sb + 1), :, n0:n1])
                pieces.append((t, n0))
            fts.append(pieces)
            if sb <= 1:
                CMAP[sb] = [max(pi for pi in range(len(bounds) - 1)
                                if bounds[pi] <= c0) for c0, _ in CH]

        pss = [ppool.tile([64, c1 - c0], f32, tag=f"ps{ci}", name=f"ps{ci}",
                          padded_shape=[64, 512]) for ci, (c0, c1) in enumerate(CH)]
        for sb in range(NSB):
            cmap = CMAP[min(sb, 1)]
            for ci, (c0, c1) in enumerate(CH):
                tl, n0 = fts[sb][cmap[ci]]
                ft3 = tl[:].rearrange("p (two n) -> p two n", two=2)
                nc.tensor.matmul(pss[ci][:, 0:c1 - c0], wt_view(sb),
                                 ft3[:, :, c0 - n0:c1 - n0],
                                 start=(sb == 0), stop=(sb == NSB - 1),
                                 perf_mode=dr)
        # PSUM -> SBUF into one tile (alternate vector/scalar; chunk stops
        # stagger across the last superblock's matmuls), then just two
        # output DMAs, one per HWDGE queue
        pt = rpool.tile([64, HPX], f32)
        for ci, (c0, c1) in enumerate(CH):
            if ci % 2 == 0:
                nc.vector.tensor_copy(pt[:, c0:c1], pss[ci][:])
            else:
                nc.scalar.copy(pt[:, c0:c1], pss[ci][:])
            if ci == 3:
                nc.sync.dma_start(p_d[:, 0:2048], pt[:, 0:2048])
        nc.scalar.dma_start(p_d[:, 2048:HPX], pt[:, 2048:HPX])
    nc.compile()
    return nc


_CACHE = {}


def _get_program():
    if "nc" not in _CACHE:
        _CACHE["nc"] = _build()
    return _CACHE["nc"]


def kernel(features, rois, W1, b1, W2, b2, _want_trace=False, _trace_kwargs=None):
    features = np.asarray(features, np.float32)
    W1 = np.asarray(W1, np.float32)
    b1 = np.asarray(b1, np.float32).reshape(64)
    W2 = np.asarray(W2, np.float32).reshape(1, 64)
    b2 = np.asarray(b2, np.float32).reshape(1)

    groups, hs, he, ws, we, recip, empty = _host_prep(rois)
    nc = _get_program()

    feat8 = features.astype(FEAT_NP)
    # wt free layout (sb, ktile, o): wt[p, 128*sb + 64*i + m] = W1[m, 256sb+128i+p]
    wt = np.ascontiguousarray(
        W1[:, :C].T.reshape(NSB, 2, 128, 64).transpose(2, 0, 1, 3)
        .reshape(128, NSB * 128)
    ).astype(FEAT_NP)

    in_maps = []
    for k in range(NCORES):
        img, hlf = k // 2, k % 2
        feat_k = feat8[img, :, hlf * HALF_ROWS:(hlf + 1) * HALF_ROWS, :]
        # [2048, 3200] -> [sb, ktile, 128p, n] -> [sb, p, (ktile, n)] rows:
        # each partition line is one contiguous 6400B DMA segment
        feat_k = (feat_k.reshape(NSB, 2, 128, HPX)
                  .transpose(0, 2, 1, 3).reshape(NSB * 128, 2 * HPX))
        in_maps.append({
            "feat": np.ascontiguousarray(feat_k),
            "wt": wt,
        })
    res = run_bass_kernel_spmd(nc, in_maps, list(range(NCORES)),
                               trace=_want_trace, **(_trace_kwargs or {}))

    # host epilogue: build per-region SATs from projected P, gather the 4
    # corners per (ROI, region), sum regions, scale by 1/area, add the
    # global-context term + b1, then tanh / W2 / softplus.
    gmean = features.mean(axis=(2, 3))          # [B, C]
    gterm = gmean @ W1[:, C:].T                 # [B, 64]
    out = np.zeros((1, R), np.float32)
    for img in range(B):
        g = groups[img]
        n = len(g)
        win = np.zeros((64, n), np.float32)
        for k in (2 * img, 2 * img + 1):
            hlf = k % 2
            pflat = np.asarray(res.results[k]["p"])         # [64, HPX]
            row0 = hlf * HALF_ROWS
            for q in range(NREG):
                rh = RHS[q]
                P = pflat[:, ROFF[q]:ROFF[q + 1]].reshape(64, rh, W)
                S = np.zeros((64, rh + 1, W + 1), np.float32)
                np.cumsum(np.cumsum(P, axis=2), axis=1, out=S[:, 1:, 1:])
                r0 = row0 + sum(RHS[:q])
                ls = np.clip(hs[g] - r0, 0, rh)
                le = np.clip(he[g] - r0, 0, rh)
                win += (S[:, le, we[g]] - S[:, ls, we[g]]
                        - S[:, le, ws[g]] + S[:, ls, ws[g]])
        pre = win * recip[g][None, :] + b1[:, None]
        pre = pre + np.where(empty[g][None, :], 0.0, gterm[img][:, None])
        h = np.tanh(pre)
        kk = W2 @ h + b2[:, None]               # [1, n]
        out[0, g] = np.log1p(np.exp(kk[0]))
    if _want_trace:
        return out, res
    return out
